# revision 3
# baseline (speedup 1.0000x reference)
"""CenterLoss forward on 8 Trainium2 NeuronCores.

loss = mean_i ||features[i] - centers[labels[i]]||^2   (N=16384, C=1000, D=512)

The reference materializes the full [N, C] distance matrix and selects one
column per row; here we instead gather each row's own center with indirect
DMAs and compute the squared distance directly -- O(N*D) work instead of
O(N*C*D).

Sharding: data-parallel over N. Each core gets 2048 rows laid out as
[128 partitions x 16 row-blocks]; centers [1000, 512] are replicated.
Features/centers are downcast to bf16 on the host (verified 5.6e-6 relative
error on the final loss); squares are accumulated in f32. Each core returns
per-partition partial sums [128, G] in f32; the host sums the 8*128*G
partials and divides by N (the "all-reduce" of the scalar loss).

HW note: an indirect DMA consumes exactly ONE dynamic row index per
partition per call (multi-index offset APs silently gather garbage or wedge
the exec unit), so each 128-row block needs its own indirect_dma_start.
"""

import numpy as np

N, C, D = 16384, 1000, 512
M = 8            # cores
NPC = N // M     # rows per core = 2048
P = 128          # SBUF partitions
J = NPC // P     # row-blocks per partition = 16
G = 4            # processing chunks per core
JB = J // G      # row-blocks per chunk
CHUNK = JB * D   # free-dim elements per chunk per partition

_prog_cache = {}


def _build():
    if "nc" in _prog_cache:
        return _prog_cache["nc"]
    import concourse.bacc as bacc
    import concourse.mybir as mybir
    from concourse import bass
    from concourse.tile import TileContext

    nc = bacc.Bacc("TRN2", target_bir_lowering=False, debug=False, num_devices=M)
    bf16 = mybir.dt.bfloat16
    f32 = mybir.dt.float32
    feats = nc.dram_tensor("features", [NPC, D], bf16, kind="ExternalInput")
    cents = nc.dram_tensor("centers", [C, D], bf16, kind="ExternalInput")
    labs = nc.dram_tensor("labels", [P, J], mybir.dt.int32, kind="ExternalInput")
    out = nc.dram_tensor("out", [P, G], f32, kind="ExternalOutput")

    with TileContext(nc) as tc:
        with (
            tc.tile_pool(name="io", bufs=3) as io_pool,
            tc.tile_pool(name="consts", bufs=1) as const_pool,
        ):
            l_tile = const_pool.tile([P, J], mybir.dt.int32)
            acc = const_pool.tile([P, G], f32)
            nc.sync.dma_start(out=l_tile[:, :], in_=labs[:, :])
            # row r = p*J + j of the shard lives at partition p, block j
            feats_ap = feats[:, :].rearrange("(p j) d -> p (j d)", p=P)
            for g in range(G):
                f_tile = io_pool.tile([P, CHUNK], bf16, tag="f")
                c_tile = io_pool.tile([P, CHUNK], bf16, tag="c")
                d_tile = io_pool.tile([P, CHUNK], bf16, tag="d")
                sq_tile = io_pool.tile([P, CHUNK], bf16, tag="sq")
                nc.sync.dma_start(
                    out=f_tile[:, :], in_=feats_ap[:, g * CHUNK : (g + 1) * CHUNK]
                )
                # one index per partition per call: block j gathers center row
                # labels[p, j] into c_tile[p, jj*D:(jj+1)*D]
                for jj in range(JB):
                    j = g * JB + jj
                    nc.gpsimd.indirect_dma_start(
                        out=c_tile[:, jj * D : (jj + 1) * D],
                        out_offset=None,
                        in_=cents[:, :],
                        in_offset=bass.IndirectOffsetOnAxis(
                            ap=l_tile[:, j : j + 1], axis=0
                        ),
                    )
                nc.vector.tensor_tensor(
                    out=d_tile[:, :],
                    in0=f_tile[:, :],
                    in1=c_tile[:, :],
                    op=mybir.AluOpType.subtract,
                )
                nc.scalar.activation(
                    out=sq_tile[:, :],
                    in_=d_tile[:, :],
                    func=mybir.ActivationFunctionType.Square,
                    accum_out=acc[:, g : g + 1],
                )
            nc.sync.dma_start(out=out[:, :], in_=acc[:, :])
    nc.compile()
    _prog_cache["nc"] = nc
    return nc


def _prepare_in_maps(features, centers, labels):
    import ml_dtypes

    bf16 = ml_dtypes.bfloat16
    feats = np.asarray(features, dtype=np.float32).astype(bf16)
    cents = np.ascontiguousarray(np.asarray(centers, dtype=np.float32).astype(bf16))
    labs = np.ascontiguousarray(
        np.asarray(labels).astype(np.int32).reshape(M, P, J)
    )
    fshard = feats.reshape(M, NPC, D)
    return [
        {
            "features": np.ascontiguousarray(fshard[m]),
            "centers": cents,
            "labels": labs[m],
        }
        for m in range(M)
    ]


def run(features, centers, labels, **spmd_kwargs):
    """Returns (loss_scalar, BassKernelResults)."""
    from concourse import bass_utils

    nc = _build()
    in_maps = _prepare_in_maps(features, centers, labels)
    res = bass_utils.run_bass_kernel_spmd(
        nc, in_maps, core_ids=list(range(M)), **spmd_kwargs
    )
    parts = np.stack([r["out"] for r in res.results])  # [M, P, G]
    total = float(parts.astype(np.float64).sum())
    loss = np.asarray(np.float32(total / N))
    return loss, res


def kernel(features, centers, labels):
    loss, _ = run(features, centers, labels)
    return loss
